# revision 1
# baseline (speedup 1.0000x reference)
"""3-layer GAT on 8 TRN2 NeuronCores via Bass/Tile.

Architecture:
- Nodes dst-sharded 12500/core, re-sorted by in-degree within shard.
- Per-layer node table in each core's DRAM: [100008, 128] bf16 rows
  [feat(64) | el | er | pad], shards of 12501 rows (row 12500 = pad row with
  el = -1e15 so padded slots contribute exp(...)=0).
- Edge gather via InstDMAGatherAnt (int16 idx): 4 windows of 25002 rows,
  per-(tile,window) rectangular slot grids, node-major [128, W, 128].
- Softmax (no max-subtraction; logits are O(1)) on ACT (Lrelu, Exp+accum) and
  DVE (weighted sum via strided-view reduce).
- BN stats via PE ones-matmul + AllReduce; inter-layer AllGather of projected
  shard tables. Layer 1 projects the full (replicated) input locally.
"""
import sys
sys.path.insert(0, "/opt/trn_rl_repo")
import os
import numpy as np
import ml_dtypes

import concourse.bass as bass
import concourse.bacc as bacc
import concourse.tile as tile
import concourse.mybir as mybir
from concourse import bass_utils
from concourse.library_config import mlp as mlp_lib
from concourse.masks import make_identity

N_NODES = 100000
N_EDGES = 1600000
D = 64
N_CORES = 8
SHARD = 12500
SHARD_P = SHARD + 1          # + pad row
N_WIN = 4
WIN_ROWS = 2 * SHARD_P       # 25002 rows per window
TAB_ROWS = N_CORES * SHARD_P # 100008
ROW = 128                    # bf16 elems per table row (256B)
NEG_SLOPE = 0.2
BN_EPS = 1e-5
P = 128
N_TILES = (SHARD + P - 1) // P          # 98 (last tile 84 nodes)
LAST_TILE_N = SHARD - (N_TILES - 1) * P  # 84
CHUNK_TILES = 5
PAD_EL = -1e15
N_LAYERS = int(os.environ.get("GAT_LAYERS", "3"))
NO_COLL = os.environ.get("GAT_NO_COLL", "0") == "1"
RAW_OUT = os.environ.get("GAT_RAW_OUT", "0") == "1"
SIM_SAFE = os.environ.get("GAT_SIM_SAFE", "0") == "1"

f32 = mybir.dt.float32
bf16 = mybir.dt.bfloat16
i16 = mybir.dt.int16


# ---------------------------------------------------------------- host side
def _preprocess(node_weight, src, dst, Ws, als, ars):
    src = np.asarray(src).astype(np.int64)
    dst = np.asarray(dst).astype(np.int64)
    deg = np.bincount(dst, minlength=N_NODES)

    # per-(node, window) incoming-edge counts; window of a src node depends
    # only on its shard (fixed), not the within-shard order.
    src_win0 = (src // SHARD) // 2
    cnt_w = np.zeros((N_NODES, N_WIN), np.int64)
    np.add.at(cnt_w, (dst, src_win0), 1)

    # per-core permutation minimizing per-(tile,window) max: lexsort by
    # (argmax window, -max window count)
    newid = np.empty(N_NODES, np.int64)
    orig_of = np.empty(N_NODES, np.int64)  # new compact (core*SHARD+rank) -> orig
    for c in range(N_CORES):
        orig = np.arange(c * SHARD, (c + 1) * SHARD)
        cw = cnt_w[orig]
        order = orig[np.lexsort((cw.argmax(1), -cw.max(1)))]
        newid[order] = c * SHARD_P + np.arange(SHARD)
        orig_of[c * SHARD: (c + 1) * SHARD] = order

    src_n = newid[src]
    dst_n = newid[dst]
    dst_core = dst // SHARD
    dst_loc = dst_n % SHARD_P  # local rank within shard [0, 12500)

    # group edges per (core, local dst), with per-window counts
    # order edges by (core, dst_loc) for grouping
    win_of_src = src_n // WIN_ROWS

    # per-core structures
    per_core = []
    Wmax = np.zeros((N_TILES, N_WIN), np.int64)
    for c in range(N_CORES):
        m = dst_core == c
        s_c = src_n[m]
        d_c = dst_loc[m]
        w_c = win_of_src[m]
        # sort by (dst_loc, window, src) for deterministic layout
        o = np.lexsort((s_c, w_c, d_c))
        s_c, d_c, w_c = s_c[o], d_c[o], w_c[o]
        # counts[dst_loc, win]
        cnt = np.zeros((SHARD, N_WIN), np.int64)
        np.add.at(cnt, (d_c, w_c), 1)
        per_core.append((s_c, d_c, w_c, cnt))
        # per-tile, per-window max
        for t in range(N_TILES):
            lo, hi = t * P, min((t + 1) * P, SHARD)
            Wmax[t] = np.maximum(Wmax[t], cnt[lo:hi].max(axis=0))

    W_tw = Wmax.astype(np.int64)  # uniform across cores

    # chunk layout
    chunks = []
    t0 = 0
    while t0 < N_TILES:
        chunks.append(list(range(t0, min(t0 + CHUNK_TILES, N_TILES))))
        t0 += CHUNK_TILES

    # per-core idx streams: for each chunk, for each window: int16 idx list
    # (column-major per tile: for t in chunk: for s < W_tw: for p in 0..127)
    idx_streams = []
    call_meta = []  # (chunk_id, win, n_idx, col16_offset) -- shared across cores
    for c in range(N_CORES):
        s_c, d_c, w_c, cnt = per_core[c]
        # slot lists: for each (dst_loc, win) the srcs (window-relative)
        # build offsets: edges sorted by (d, w) so contiguous runs
        # compute run starts per (d, w)
        key = d_c * N_WIN + w_c
        # positions of each (d,w) run
        run_start = np.zeros(SHARD * N_WIN + 1, np.int64)
        np.add.at(run_start, key + 1, 1)
        run_start = np.cumsum(run_start)
        stream = []
        meta = []
        for ci, ch in enumerate(chunks):
            for w in range(N_WIN):
                win_pad = 12500  # window-relative pad row (first shard's pad)
                vals = []
                for t in ch:
                    Wt = int(W_tw[t, w])
                    if Wt == 0:
                        continue
                    n_in_tile = P if t < N_TILES - 1 else LAST_TILE_N
                    block = np.full((Wt, P), win_pad, np.int64)
                    for p in range(n_in_tile):
                        d_l = t * P + p
                        a = run_start[d_l * N_WIN + w]
                        b = run_start[d_l * N_WIN + w + 1]
                        k = b - a
                        if k:
                            block[:k, p] = s_c[a:b] - w * WIN_ROWS
                    vals.append(block.reshape(-1))
                if not vals:
                    if c == 0:
                        meta.append((ci, w, 0, 0))
                    continue
                v = np.concatenate(vals)
                n_idx = v.size  # multiple of 128
                # int16 wrap into 16 partitions, cols n/16, replicate x8
                v16 = v.astype(np.int16).reshape(-1, 16).T  # [16, n/16]
                stream.append(np.tile(v16, (8, 1)))  # [128, n/16]
                if c == 0:
                    meta.append((ci, w, n_idx, 0))
        idx_cat = np.concatenate(stream, axis=1)  # [128, C16]
        idx_streams.append(np.ascontiguousarray(idx_cat))
        if c == 0:
            # fill col16 offsets
            off = 0
            call_meta = []
            k = 0
            for ci, ch in enumerate(chunks):
                for w in range(N_WIN):
                    _, _, n_idx, _ = meta[k]
                    call_meta.append((ci, w, n_idx, off))
                    off += n_idx // 16
                    k += 1

    # layer-1 transposed, permuted, padded input  [64, TAB_ROWS] f32
    nwT = np.zeros((D, TAB_ROWS), np.float32)
    nw = np.asarray(node_weight, np.float32)
    for c in range(N_CORES):
        rows = orig_of[c * SHARD: (c + 1) * SHARD]
        nwT[:, c * SHARD_P: c * SHARD_P + SHARD] = nw[rows].T

    # per-core own-shard transposed input [64, SHARD] (for er matmul path it
    # is just a slice of nwT; pass per-core)
    own_hT = [np.ascontiguousarray(nwT[:, c * SHARD_P: c * SHARD_P + SHARD])
              for c in range(N_CORES)]

    # Wstack per layer [64, 66] = [W | W@al | W@ar]
    wstk = np.concatenate(
        [np.concatenate([Ws[l], (Ws[l] @ als[l])[:, None], (Ws[l] @ ars[l])[:, None]],
                        axis=1)[None] for l in range(3)], axis=0
    ).astype(np.float32)  # [3, 64, 66]

    return dict(
        W_tw=W_tw, chunks=chunks, call_meta=call_meta,
        idx_streams=idx_streams, nwT=nwT, own_hT=own_hT, wstk=wstk,
        orig_of=orig_of,
    )


# ---------------------------------------------------------------- device side
def _build_nc(W_tw, chunks, call_meta, C16):
    nc = bacc.Bacc("TRN2", target_bir_lowering=False, debug=False,
                   num_devices=N_CORES)

    nwT_in = nc.dram_tensor("nwT", [D, TAB_ROWS], f32, kind="ExternalInput")
    ownT_in = nc.dram_tensor("ownT", [D, SHARD], f32, kind="ExternalInput")
    idx_in = nc.dram_tensor("idx", [P, C16], i16, kind="ExternalInput")
    wstk_in = nc.dram_tensor("wstk", [3, D, 66], f32, kind="ExternalInput")
    bnp_in = nc.dram_tensor("bnp", [3, 3, D], f32, kind="ExternalInput")  # b,g,beta
    out_t = nc.dram_tensor("out", [SHARD, D], f32, kind="ExternalOutput")

    rg = [list(range(N_CORES))]
    nc.gpsimd.load_library(mlp_lib)

    with tile.TileContext(nc) as tc:
        with (
            tc.tile_pool(name="const", bufs=1) as constp,
            tc.tile_pool(name="gbuf", bufs=6) as gbuf,
            tc.tile_pool(name="idxb", bufs=4) as idxb,
            tc.tile_pool(name="small", bufs=4) as small,
            tc.tile_pool(name="acc", bufs=1) as accp,
            tc.tile_pool(name="acc2", bufs=3) as accp2,
            tc.tile_pool(name="ps", bufs=2, space="PSUM") as ps,
            tc.tile_pool(name="pstat", bufs=1, space="PSUM") as pstat,
            tc.tile_pool(name="dram", bufs=1, space="DRAM") as dram,
        ):
            ident = constp.tile([P, P], f32)
            make_identity(nc, ident[:])
            ones_col = constp.tile([P, 1], f32)
            nc.vector.memset(ones_col[:], 1.0)

            # weights resident
            wstk_t = constp.tile([D, 3 * 66], f32)
            nc.sync.dma_start(out=wstk_t[:].rearrange("k (l n) -> k l n", n=66), in_=wstk_in[:, :, :].rearrange("l k n -> k l n"))
            bnp_t = constp.tile([P, 9 * D], f32)  # broadcast rows [128, 3*3*64]
            nc.sync.dma_start(out=bnp_t[:], in_=bnp_in[:, :, :].rearrange("l k n -> (l k n)")[None, :].to_broadcast([P, 9 * D]))

            # pad row template [1, 128] bf16: zeros except el=-1e15
            padrow = constp.tile([1, ROW], bf16)
            nc.vector.memset(padrow[:], 0.0)
            nc.vector.memset(padrow[:, 64:66], PAD_EL)

            # er for own shard, per tile column [128, N_TILES] f32
            er_sb = constp.tile([P, N_TILES], f32)
            # out tiles resident [128, N_TILES*64] f32
            out_sb = accp.tile([P, N_TILES * D], f32)

            t0_w = []
            for w in range(N_WIN):
                tbl = dram.tile([WIN_ROWS, ROW], bf16, tag=f"t0w{w}", name=f"t0w{w}")
                t0_w.append(tbl)
            tab1 = dram.tile([TAB_ROWS, ROW], bf16, name="tab1")
            tab2 = dram.tile([TAB_ROWS, ROW], bf16, name="tab2")
            tables = [t0_w,
                      [tab1[w * WIN_ROWS:(w + 1) * WIN_ROWS, :] for w in range(N_WIN)],
                      [tab2[w * WIN_ROWS:(w + 1) * WIN_ROWS, :] for w in range(N_WIN)]]
            ag_tabs = [None, tab1, tab2]
            shard_buf = dram.tile([SHARD_P, ROW], bf16)
            stats_dram_in = dram.tile([D, 2], f32)
            stats_dram_out = dram.tile([D, 2], f32)
            bcast_dram = dram.tile([3, D], f32)

            # ---------------- layer-1: full local projection ----------------
            # own-shard er for layer 1: er = ownT.T @ War1
            for t in range(N_TILES):
                m = P if t < N_TILES - 1 else LAST_TILE_N
                hT = gbuf.tile([D, P], f32, tag="l1e")
                nc.sync.dma_start(out=hT[:, :m], in_=ownT_in[:, t * P: t * P + m])
                pt = ps.tile([P, 1], f32, tag="mm")
                nc.tensor.matmul(out=pt[:m, :], lhsT=hT[:, :m],
                                 rhs=wstk_t[:, 65:66], start=True, stop=True)
                nc.vector.tensor_copy(out=er_sb[:m, t:t + 1], in_=pt[:m, :])

            # process 4 tiles per group: load nwT [64, 512], 4 matmuls,
            # copy to bf16 staging [128, 4*128], strided DMA out.
            GT = 8
            n_groups = (TAB_ROWS + GT * P - 1) // (GT * P)
            for g in range(n_groups):
                col0 = g * GT * P
                ncols = min(GT * P, TAB_ROWS - col0)
                nj = (ncols + P - 1) // P
                hT = gbuf.tile([D, GT * P], f32, tag="l1h")
                nc.sync.dma_start(out=hT[:, :ncols], in_=nwT_in[:, col0:col0 + ncols])
                stage = gbuf.tile([P, GT * 66], bf16, tag="l1s")
                for j in range(nj):
                    m = min(P, ncols - j * P)
                    pt = ps.tile([P, 66], f32, tag="mm")
                    nc.tensor.matmul(
                        out=pt[:m, :], lhsT=hT[:, j * P: j * P + m],
                        rhs=wstk_t[:, 0:66], start=True, stop=True,
                    )
                    nc.scalar.copy(out=stage[:m, j * 66:(j + 1) * 66], in_=pt[:m, :])
                w0 = col0 // WIN_ROWS
                w1 = (col0 + ncols - 1) // WIN_ROWS
                weng = nc.gpsimd if (g % 2 == 0) else nc.sync
                if ncols == GT * P and w0 == w1 and (col0 % WIN_ROWS) % P == 0:
                    r0 = col0 - w0 * WIN_ROWS
                    weng.dma_start(
                        out=tables[0][w0][r0:r0 + ncols, 0:66].rearrange(
                            "(j p) n -> p j n", p=P),
                        in_=stage[:, :].rearrange("p (j n) -> p j n", n=66),
                    )
                else:
                    for j in range(nj):
                        m = min(P, ncols - j * P)
                        rj = col0 + j * P
                        wj = rj // WIN_ROWS
                        if rj + m <= (wj + 1) * WIN_ROWS:
                            weng.dma_start(
                                out=tables[0][wj][rj - wj * WIN_ROWS: rj - wj * WIN_ROWS + m, 0:66],
                                in_=stage[:m, j * 66:(j + 1) * 66],
                            )
                        else:
                            k = (wj + 1) * WIN_ROWS - rj
                            nc.sync.dma_start(
                                out=tables[0][wj][rj - wj * WIN_ROWS: rj - wj * WIN_ROWS + k, 0:66],
                                in_=stage[:k, j * 66:(j + 1) * 66],
                            )
                            nc.sync.dma_start(
                                out=tables[0][wj + 1][0:m - k, 0:66],
                                in_=stage[k:m, j * 66:(j + 1) * 66],
                            )
            # pad rows of table 0 (8 shards)
            for sh in range(N_CORES):
                g_r = sh * SHARD_P + SHARD
                w_r = g_r // WIN_ROWS
                nc.sync.dma_start(
                    out=tables[0][w_r][g_r - w_r * WIN_ROWS: g_r - w_r * WIN_ROWS + 1, :],
                    in_=padrow[:, :],
                )

            # ---------------- per-layer gather + aggregate ----------------
            for l in range(N_LAYERS):
                table = tables[l]  # list of 4 window tiles
                stat_s = pstat.tile([D, 1], f32, tag="stat_s")
                stat_q = pstat.tile([D, 1], f32, tag="stat_q")
                for ci, ch in enumerate(chunks):
                    nch = len(ch)
                    s4c = small.tile([P, nch * N_WIN], f32, tag="s4c")
                    acc4c = accp2.tile([P, nch * N_WIN * D], f32, tag="acc4c")
                    for w in range(N_WIN):
                        meta = call_meta[ci * N_WIN + w]
                        _, _, n_idx, off16 = meta
                        if n_idx == 0:
                            continue
                        it = idxb.tile([P, n_idx // 16], i16, tag="idx")
                        nc.sync.dma_start(out=it[:], in_=idx_in[:, off16: off16 + n_idx // 16])
                        gt = gbuf.tile([P, (n_idx // P) * ROW], bf16, tag="g")
                        nc.gpsimd.dma_gather(
                            out_ap=gt[:].rearrange("p (c r) -> p c r", r=ROW),
                            in_ap=table[w][:, :] if l == 0 else table[w],
                            idxs_ap=it[:, :],
                            num_idxs=n_idx,
                            num_idxs_reg=n_idx,
                            elem_size=ROW,
                            single_packet=False,
                        )
                        o = 0
                        for ti, t in enumerate(ch):
                            Wt = int(W_tw[t, w])
                            if Wt == 0:
                                continue
                            g3 = gt[:].rearrange("p (c r) -> p c r", r=ROW)
                            el_v = g3[:, o:o + Wt, 64:65].rearrange("p w o -> p (w o)")
                            ft_v = g3[:, o:o + Wt, 0:64]
                            e_t = small.tile([P, Wt], f32, tag="e")
                            if SIM_SAFE:
                                nc.scalar.activation(
                                    out=e_t[:], in_=el_v,
                                    func=mybir.ActivationFunctionType.Identity,
                                    bias=er_sb[:, t:t + 1], scale=1.0,
                                )
                                e_s = small.tile([P, Wt], f32, tag="es")
                                nc.vector.tensor_scalar(
                                    out=e_s[:], in0=e_t[:], scalar1=NEG_SLOPE,
                                    scalar2=None, op0=mybir.AluOpType.mult)
                                nc.vector.tensor_tensor(
                                    out=e_t[:], in0=e_t[:], in1=e_s[:],
                                    op=mybir.AluOpType.max)
                            else:
                                nc.scalar.activation(
                                    out=e_t[:], in_=el_v,
                                    func=mybir.ActivationFunctionType.Prelu,
                                    bias=er_sb[:, t:t + 1], scale=1.0,
                                    alpha=NEG_SLOPE,
                                )
                            ex_t = small.tile([P, Wt], f32, tag="x")
                            nc.scalar.activation(
                                out=ex_t[:], in_=e_t[:],
                                func=mybir.ActivationFunctionType.Exp,
                                accum_out=s4c[:, ti * N_WIN + w: ti * N_WIN + w + 1],
                            )
                            wf = small.tile([P, Wt * D], f32, tag="wf")
                            nc.vector.tensor_tensor(
                                out=wf[:].rearrange("p (w d) -> p w d", d=D),
                                in0=ft_v,
                                in1=ex_t[:].unsqueeze(2).to_broadcast([P, Wt, D]),
                                op=mybir.AluOpType.mult,
                            )
                            nc.vector.tensor_reduce(
                                out=acc4c[:, (ti * N_WIN + w) * D:(ti * N_WIN + w + 1) * D],
                                in_=wf[:].rearrange("p (w d) -> p d w", d=D),
                                axis=mybir.AxisListType.X, op=mybir.AluOpType.add,
                            )
                            o += Wt
                    # per tile: combine windows (zero-width windows left uninit:
                    # exclude by summing only active lanes via host-known mask)
                    for ti, t in enumerate(ch):
                        act_ws = [w for w in range(N_WIN) if W_tw[t, w] > 0]
                        base = ti * N_WIN
                        ssum = small.tile([P, 1], f32, tag="ss")
                        if len(act_ws) == N_WIN:
                            nc.vector.tensor_reduce(
                                out=ssum[:], in_=s4c[:, base:base + N_WIN],
                                axis=mybir.AxisListType.X, op=mybir.AluOpType.add)
                        else:
                            nc.vector.tensor_copy(out=ssum[:], in_=s4c[:, base + act_ws[0]: base + act_ws[0] + 1])
                            for w in act_ws[1:]:
                                nc.vector.tensor_tensor(
                                    out=ssum[:], in0=ssum[:],
                                    in1=s4c[:, base + w: base + w + 1],
                                    op=mybir.AluOpType.add)
                        rinv = small.tile([P, 1], f32, tag="ri")
                        nc.vector.reciprocal(out=rinv[:], in_=ssum[:])
                        aggr = small.tile([P, D], f32, tag="ag")
                        if len(act_ws) == N_WIN:
                            nc.vector.tensor_reduce(
                                out=aggr[:],
                                in_=acc4c[:, base * D:(base + N_WIN) * D].rearrange(
                                    "p (w d) -> p d w", d=D),
                                axis=mybir.AxisListType.X, op=mybir.AluOpType.add)
                        else:
                            nc.vector.tensor_copy(
                                out=aggr[:],
                                in_=acc4c[:, (base + act_ws[0]) * D:(base + act_ws[0] + 1) * D])
                            for w in act_ws[1:]:
                                nc.vector.tensor_tensor(
                                    out=aggr[:], in0=aggr[:],
                                    in1=acc4c[:, (base + w) * D:(base + w + 1) * D],
                                    op=mybir.AluOpType.add)
                        ot = out_sb[:, t * D:(t + 1) * D]
                        nc.scalar.activation(
                            out=aggr[:], in_=aggr[:],
                            func=mybir.ActivationFunctionType.Copy,
                            scale=rinv[:, :])
                        nc.vector.tensor_tensor(
                            out=ot, in0=aggr[:], in1=bnp_t[:, (3 * l) * D:(3 * l + 1) * D],
                            op=mybir.AluOpType.add)
                        m = P if t < N_TILES - 1 else LAST_TILE_N
                        sq = small.tile([P, D], f32, tag="sq")
                        nc.scalar.activation(out=sq[:], in_=ot,
                                             func=mybir.ActivationFunctionType.Square)
                        first = (ci == 0 and t == ch[0])
                        last = (t == N_TILES - 1)
                        nc.tensor.matmul(out=stat_s[:, :], lhsT=ot[:m, :],
                                         rhs=ones_col[:m, :],
                                         start=first, stop=last)
                        nc.tensor.matmul(out=stat_q[:, :], lhsT=sq[:m, :],
                                         rhs=ones_col[:m, :],
                                         start=first, stop=last)

                # ---- BN stats all-reduce ----
                stat_sb = small.tile([D, 2], f32, tag="stc")
                nc.vector.tensor_copy(out=stat_sb[:, 0:1], in_=stat_s[:])
                nc.vector.tensor_copy(out=stat_sb[:, 1:2], in_=stat_q[:])
                nc.gpsimd.dma_start(out=stats_dram_in[:], in_=stat_sb[:])
                if not NO_COLL:
                    nc.gpsimd.collective_compute(
                        "AllReduce", mybir.AluOpType.add, replica_groups=rg,
                        ins=[stats_dram_in.opt()], outs=[stats_dram_out.opt()],
                    )
                stat_g = small.tile([D, 2], f32, tag="stg")
                nc.sync.dma_start(out=stat_g[:], in_=(stats_dram_in if NO_COLL else stats_dram_out)[:])
                # mu = s/N ; var = sq/N - mu^2 ; rstd = 1/sqrt(var+eps)
                mu = small.tile([D, 1], f32, tag="mu")
                nc.vector.tensor_scalar(out=mu[:], in0=stat_g[:, 0:1],
                                        scalar1=1.0 / N_NODES, scalar2=None,
                                        op0=mybir.AluOpType.mult)
                musq = small.tile([D, 1], f32, tag="musq")
                nc.scalar.activation(out=musq[:], in_=mu[:],
                                     func=mybir.ActivationFunctionType.Square)
                var = small.tile([D, 1], f32, tag="var")
                nc.vector.tensor_scalar(out=var[:], in0=stat_g[:, 1:2],
                                        scalar1=1.0 / N_NODES, scalar2=None,
                                        op0=mybir.AluOpType.mult)
                nc.vector.tensor_tensor(out=var[:], in0=var[:], in1=musq[:],
                                        op=mybir.AluOpType.subtract)
                nc.vector.tensor_scalar(out=var[:], in0=var[:], scalar1=BN_EPS,
                                        scalar2=None, op0=mybir.AluOpType.add)
                sd = small.tile([D, 1], f32, tag="sd")
                nc.scalar.activation(out=sd[:], in_=var[:],
                                     func=mybir.ActivationFunctionType.Sqrt)
                rstd = small.tile([D, 1], f32, tag="rstd")
                nc.vector.reciprocal(out=rstd[:], in_=sd[:])
                # column vectors for dim-major BN: g/beta as [D,1]
                gcol = small.tile([D, 1], f32, tag="gc")
                nc.sync.dma_start(out=gcol[:], in_=bnp_in[l, 1, :][:, None])
                bcol = small.tile([D, 1], f32, tag="bc")
                nc.sync.dma_start(out=bcol[:], in_=bnp_in[l, 2, :][:, None])
                grs = small.tile([D, 1], f32, tag="grs")
                nc.vector.tensor_tensor(out=grs[:], in0=gcol[:], in1=rstd[:],
                                        op=mybir.AluOpType.mult)
                negmu = small.tile([D, 1], f32, tag="nmu")
                nc.vector.tensor_scalar(out=negmu[:], in0=mu[:], scalar1=-1.0,
                                        scalar2=None, op0=mybir.AluOpType.mult)

                if l < N_LAYERS - 1:
                    # pass 2: transpose out tiles, BN+ELU, project, write shard_buf
                    for t in range(N_TILES):
                        m = P if t < N_TILES - 1 else LAST_TILE_N
                        ot = out_sb[:, t * D:(t + 1) * D]
                        pT = ps.tile([D, P], f32, tag="pT")
                        nc.tensor.transpose(out=pT[:, :m], in_=ot[:m, :], identity=ident[:m, :m])
                        z = small.tile([D, P], f32, tag="z")
                        # z = (x - mu) * grs + beta
                        nc.vector.tensor_scalar(
                            out=z[:, :m], in0=pT[:, :m], scalar1=negmu[:, :],
                            scalar2=grs[:, :], op0=mybir.AluOpType.add,
                            op1=mybir.AluOpType.mult)
                        nc.vector.tensor_scalar(
                            out=z[:, :m], in0=z[:, :m], scalar1=bcol[:, :],
                            scalar2=None, op0=mybir.AluOpType.add)
                        # ELU: relu(z) + min(exp(z)-1, 0)
                        ez = small.tile([D, P], f32, tag="ez")
                        nc.scalar.activation(out=ez[:, :m], in_=z[:, :m],
                                             func=mybir.ActivationFunctionType.Exp)
                        nc.vector.tensor_scalar(
                            out=ez[:, :m], in0=ez[:, :m], scalar1=-1.0, scalar2=0.0,
                            op0=mybir.AluOpType.add, op1=mybir.AluOpType.min)
                        nc.vector.tensor_scalar(
                            out=z[:, :m], in0=z[:, :m], scalar1=0.0, scalar2=None,
                            op0=mybir.AluOpType.max)
                        h2 = small.tile([D, P], f32, tag="h2")
                        nc.vector.tensor_tensor(out=h2[:, :m], in0=z[:, :m],
                                                in1=ez[:, :m], op=mybir.AluOpType.add)
                        # project with next layer weights
                        pj = ps.tile([P, 66], f32, tag="mm")
                        nc.tensor.matmul(out=pj[:m, :], lhsT=h2[:, :m],
                                         rhs=wstk_t[:, (l + 1) * 66:(l + 2) * 66],
                                         start=True, stop=True)
                        stg = small.tile([P, 66], bf16, tag="stg2")
                        nc.scalar.copy(out=stg[:m, :], in_=pj[:m, :])
                        nc.gpsimd.dma_start(out=shard_buf[t * P: t * P + m, 0:66],
                                          in_=stg[:m, :])
                        nc.vector.tensor_copy(out=er_sb[:m, t:t + 1], in_=pj[:m, 65:66])
                    nc.sync.dma_start(out=shard_buf[SHARD:SHARD + 1, :], in_=padrow[:, :])
                    if not NO_COLL:
                        nc.gpsimd.collective_compute(
                            "AllGather", mybir.AluOpType.bypass, replica_groups=rg,
                            ins=[shard_buf.opt()], outs=[ag_tabs[l + 1].opt()],
                        )
                elif RAW_OUT:
                    for t in range(N_TILES):
                        m = P if t < N_TILES - 1 else LAST_TILE_N
                        nc.sync.dma_start(out=out_t[t * P:t * P + m, :],
                                          in_=out_sb[:m, t * D:(t + 1) * D])
                else:
                    # final BN in node-major; need row-broadcast vectors
                    nc.gpsimd.dma_start(out=bcast_dram[0, :], in_=negmu[:, 0])
                    nc.gpsimd.dma_start(out=bcast_dram[1, :], in_=grs[:, 0])
                    nc.gpsimd.dma_start(out=bcast_dram[2, :], in_=bcol[:, 0])
                    brow = small.tile([P, 3 * D], f32, tag="brow")
                    nc.sync.dma_start(
                        out=brow[:],
                        in_=bcast_dram[:, :].rearrange("a b -> (a b)")[None, :].to_broadcast([P, 3 * D]))
                    for t in range(N_TILES):
                        m = P if t < N_TILES - 1 else LAST_TILE_N
                        ot = out_sb[:, t * D:(t + 1) * D]
                        y = small.tile([P, D], f32, tag="y")
                        nc.vector.tensor_tensor(out=y[:m, :], in0=ot[:m, :],
                                                in1=brow[:m, 0:D], op=mybir.AluOpType.add)
                        nc.vector.tensor_tensor(out=y[:m, :], in0=y[:m, :],
                                                in1=brow[:m, D:2 * D], op=mybir.AluOpType.mult)
                        nc.vector.tensor_tensor(out=y[:m, :], in0=y[:m, :],
                                                in1=brow[:m, 2 * D:3 * D], op=mybir.AluOpType.add)
                        nc.sync.dma_start(out=out_t[t * P:t * P + m, :], in_=y[:m, :])

    nc.compile()
    return nc


_CACHE = {}


def kernel(node_weight, edge_weight, src, dst,
           W1, al1, ar1, b1, g1, beta1,
           W2, al2, ar2, b2, g2, beta2,
           W3, al3, ar3, b3, g3, beta3):
    Ws = [np.asarray(W1, np.float32), np.asarray(W2, np.float32), np.asarray(W3, np.float32)]
    als = [np.asarray(al1, np.float32), np.asarray(al2, np.float32), np.asarray(al3, np.float32)]
    ars = [np.asarray(ar1, np.float32), np.asarray(ar2, np.float32), np.asarray(ar3, np.float32)]
    pre = _preprocess(node_weight, src, dst, Ws, als, ars)

    C16 = pre["idx_streams"][0].shape[1]
    key = ("nc", C16, N_LAYERS, NO_COLL, RAW_OUT, tuple(pre["W_tw"].reshape(-1).tolist()))
    if key not in _CACHE:
        _CACHE[key] = _build_nc(pre["W_tw"], pre["chunks"], pre["call_meta"], C16)
    nc = _CACHE[key]

    bnp = np.stack([
        np.stack([np.asarray(b, np.float32), np.asarray(g, np.float32),
                  np.asarray(be, np.float32)])
        for b, g, be in ((b1, g1, beta1), (b2, g2, beta2), (b3, g3, beta3))
    ])  # [3, 3, 64]

    in_maps = []
    for c in range(N_CORES):
        in_maps.append({
            "nwT": pre["nwT"],
            "ownT": pre["own_hT"][c],
            "idx": pre["idx_streams"][c],
            "wstk": pre["wstk"],
            "bnp": bnp,
        })
    res = bass_utils.run_bass_kernel_spmd(nc, in_maps, core_ids=list(range(N_CORES)))

    out = np.empty((N_NODES, D), np.float32)
    for c in range(N_CORES):
        rows = pre["orig_of"][c * SHARD: (c + 1) * SHARD]
        out[rows] = res.results[c]["out"]
    return out



# revision 29
# speedup vs baseline: 1.4302x; 1.4302x over previous
"""3-layer GAT on 8 TRN2 NeuronCores via Bass/Tile — v2.

Architecture (per core, dst-sharded 12500 nodes):
- Host precomputes layer-1 projection: table0 [100008, 128] bf16 rows
  [feat(64) | one(1)@64 | el@65 | pad] uploaded as input (no device l1 pass).
  The ones column makes the weighted-feature reduce also produce the softmax
  denominator (col 64 of the reduce output) for free.
- Per-layer edge gather via InstDMAGatherAnt (int16 idx, 4 windows of 25002
  rows), idx streams SBUF-resident (identical across layers, loaded once).
- Slot grids: per (tile=128 dsts, window) rectangles, heights = cross-core max
  (shared compiled module); node order lexsort(-2ndmax, argmax, -max).
- e = lrelu(el+er) per (tile,window) on ACT (Prelu, bias=er ptr); exp once per
  chunk; weighted mult on DVE; per-(tile,window) reduce split DVE/Pool.
- BN: sums+sumsq via PE ones-matmuls (square on ACT per chunk), AllReduce;
  bulk node-major BN(+ELU) between layers; per-tile transpose+project (PE);
  AllGather of projected shard tables. b_l dropped (cancels in train-mode BN).
"""
import sys
sys.path.insert(0, "/opt/trn_rl_repo")
import os
import numpy as np
import ml_dtypes

import concourse.bass as bass
import concourse.bacc as bacc
import concourse.tile as tile
import concourse.mybir as mybir
from concourse import bass_utils
from concourse.library_config import mlp as mlp_lib
from concourse.masks import make_identity

N_NODES = 100000
N_EDGES = 1600000
D = 64
N_CORES = 8
SHARD = 12500
SHARD_P = SHARD + 1            # + pad row
N_WIN = 4
WIN_ROWS = 2 * SHARD_P         # 25002 rows per window
TAB_ROWS = N_CORES * SHARD_P   # 100008
ROW = 128                      # bf16 elems per table row (256B)
C_ONE = 64                     # ones column
C_EL = 65                      # el column
NEG_SLOPE = 0.2
BN_EPS = 1e-5
P = 128
N_TILES = (SHARD + P - 1) // P            # 98
LAST_TILE_N = SHARD - (N_TILES - 1) * P   # 84
PAD_EL = -1e15

CAP = int(os.environ.get("GAT_CAP", "64"))          # max slot-cols per (chunk,window)
MAX_CT = int(os.environ.get("GAT_MAXCT", "8"))      # max tiles per chunk
GBUFS = int(os.environ.get("GAT_GBUFS", "4"))
MUL_POOL_MOD = int(os.environ.get("GAT_MULPOOL", "0"))  # every k-th mult on Pool (0=off)
N_LAYERS = int(os.environ.get("GAT_LAYERS", "3"))
NO_COLL = os.environ.get("GAT_NO_COLL", "0") == "1"
SIM_SAFE = os.environ.get("GAT_SIM_SAFE", "0") == "1"

f32 = mybir.dt.float32
bf16 = mybir.dt.bfloat16
i16 = mybir.dt.int16


# ---------------------------------------------------------------- host side
def _preprocess(node_weight, src, dst, Ws, als, ars):
    src = np.asarray(src).astype(np.int64)
    dst = np.asarray(dst).astype(np.int64)

    # window of a src node = its shard pair (fixed by dst-sharding)
    src_win0 = (src // SHARD) // 2
    cnt_w = np.zeros((N_NODES, N_WIN), np.int64)
    np.add.at(cnt_w, (dst, src_win0), 1)

    # per-core permutation: lexsort(-2nd-max, argmax, -max)
    newid = np.empty(N_NODES, np.int64)
    orig_of = np.empty(N_NODES, np.int64)  # compact (core*SHARD+rank) -> orig
    for c in range(N_CORES):
        orig = np.arange(c * SHARD, (c + 1) * SHARD)
        cw = cnt_w[orig]
        s = np.sort(cw, axis=1)[:, ::-1]
        order = orig[np.lexsort((-s[:, 1], cw.argmax(1), -s[:, 0]))]
        newid[order] = c * SHARD_P + np.arange(SHARD)
        orig_of[c * SHARD: (c + 1) * SHARD] = order

    src_n = newid[src]
    dst_core = dst // SHARD
    dst_loc = newid[dst] % SHARD_P
    win_of_src = src_n // WIN_ROWS

    # per-core grouped edges + cross-core W_tw
    per_core = []
    Wmax = np.zeros((N_TILES, N_WIN), np.int64)
    for c in range(N_CORES):
        m = dst_core == c
        s_c, d_c, w_c = src_n[m], dst_loc[m], win_of_src[m]
        o = np.lexsort((s_c, w_c, d_c))
        s_c, d_c, w_c = s_c[o], d_c[o], w_c[o]
        cnt = np.zeros((SHARD, N_WIN), np.int64)
        np.add.at(cnt, (d_c, w_c), 1)
        per_core.append((s_c, d_c, w_c, cnt))
        for t in range(N_TILES):
            lo, hi = t * P, min((t + 1) * P, SHARD)
            Wmax[t] = np.maximum(Wmax[t], cnt[lo:hi].max(axis=0))
    W_tw = Wmax

    # chunk layout: greedy, per-window slot-cols <= CAP, tiles <= MAX_CT
    chunks = []
    cur = []
    acc_w = np.zeros(N_WIN, np.int64)
    for t in range(N_TILES):
        if cur and (len(cur) >= MAX_CT or np.any(acc_w + W_tw[t] > CAP)):
            chunks.append(cur)
            cur, acc_w = [], np.zeros(N_WIN, np.int64)
        cur.append(t)
        acc_w += W_tw[t]
    if cur:
        chunks.append(cur)

    # idx streams per core; call_meta shared: (ci, w, n_idx, off16)
    idx_streams = []
    call_meta = []
    for c in range(N_CORES):
        s_c, d_c, w_c, cnt = per_core[c]
        key = d_c * N_WIN + w_c
        run_start = np.zeros(SHARD * N_WIN + 1, np.int64)
        np.add.at(run_start, key + 1, 1)
        run_start = np.cumsum(run_start)
        stream = []
        meta = []
        for ci, ch in enumerate(chunks):
            for w in range(N_WIN):
                win_pad = 12500  # window-relative pad row of first shard in win
                vals = []
                for t in ch:
                    Wt = int(W_tw[t, w])
                    if Wt == 0:
                        continue
                    n_in_tile = P if t < N_TILES - 1 else LAST_TILE_N
                    block = np.full((Wt, P), win_pad, np.int64)
                    for p in range(n_in_tile):
                        d_l = t * P + p
                        a = run_start[d_l * N_WIN + w]
                        b = run_start[d_l * N_WIN + w + 1]
                        if b > a:
                            block[:b - a, p] = s_c[a:b] - w * WIN_ROWS
                    vals.append(block.reshape(-1))
                if not vals:
                    if c == 0:
                        meta.append((ci, w, 0))
                    continue
                v = np.concatenate(vals)
                v16 = v.astype(np.int16).reshape(-1, 16).T   # [16, n/16]
                stream.append(np.tile(v16, (8, 1)))          # [128, n/16]
                if c == 0:
                    meta.append((ci, w, v.size))
        idx_cat = np.concatenate(stream, axis=1)
        idx_streams.append(np.ascontiguousarray(idx_cat))
        if c == 0:
            off = 0
            for ci, w, n_idx in meta:
                call_meta.append((ci, w, n_idx, off))
                off += n_idx // 16

    # host layer-1 projection -> table0 + per-core er1
    nw = np.asarray(node_weight, np.float32)
    feat1 = nw @ Ws[0]
    el1 = feat1 @ als[0]
    er1 = feat1 @ ars[0]
    tab0 = np.zeros((TAB_ROWS, ROW), np.float32)
    er1_sb = np.zeros((N_CORES, P, N_TILES), np.float32)
    for c in range(N_CORES):
        rows = orig_of[c * SHARD: (c + 1) * SHARD]
        base = c * SHARD_P
        tab0[base: base + SHARD, 0:D] = feat1[rows]
        tab0[base: base + SHARD, C_ONE] = 1.0
        tab0[base: base + SHARD, C_EL] = el1[rows]
        tab0[base + SHARD, C_EL] = PAD_EL  # pad row
        er_full = np.zeros(N_TILES * P, np.float32)
        er_full[:SHARD] = er1[rows]
        er1_sb[c] = er_full.reshape(N_TILES, P).T
    tab0 = tab0.astype(ml_dtypes.bfloat16)

    # wstk for layers 2,3: [W | 0 | W@al | W@ar]  [2, 64, 67]
    wstk = np.stack([
        np.concatenate([Ws[l], np.zeros((D, 1), np.float32),
                        (Ws[l] @ als[l])[:, None], (Ws[l] @ ars[l])[:, None]],
                       axis=1)
        for l in (1, 2)
    ]).astype(ml_dtypes.bfloat16)

    return dict(W_tw=W_tw, chunks=chunks, call_meta=call_meta,
                idx_streams=idx_streams, tab0=tab0, er1_sb=er1_sb,
                wstk=wstk, orig_of=orig_of)


# ---------------------------------------------------------------- device side
def _build_nc(W_tw, chunks, call_meta, C16):
    nc = bacc.Bacc("TRN2", target_bir_lowering=False, debug=False,
                   num_devices=N_CORES)

    tab0_in = nc.dram_tensor("tab0", [TAB_ROWS, ROW], bf16, kind="ExternalInput")
    idx_in = nc.dram_tensor("idx", [P, C16], i16, kind="ExternalInput")
    er1_in = nc.dram_tensor("er1", [P, N_TILES], f32, kind="ExternalInput")
    wstk_in = nc.dram_tensor("wstk", [2, D, 67], bf16, kind="ExternalInput")
    bnp_in = nc.dram_tensor("bnp", [3, 2, D], f32, kind="ExternalInput")  # g, beta
    out_t = nc.dram_tensor("out", [P, N_TILES * D], f32, kind="ExternalOutput")

    rg = [list(range(N_CORES))]
    nc.gpsimd.load_library(mlp_lib)

    n_chunks = len(chunks)
    meta_by_cw = {(ci, w): (n, o) for ci, w, n, o in call_meta}

    with tile.TileContext(nc) as tc:
        with (
            tc.tile_pool(name="const", bufs=1) as constp,
            tc.tile_pool(name="gbuf", bufs=GBUFS) as gbuf,
            tc.tile_pool(name="xmp", bufs=2) as xmp,
            tc.tile_pool(name="wfp", bufs=2) as wfp,
            tc.tile_pool(name="small", bufs=4) as small,
            tc.tile_pool(name="accp", bufs=2) as accp,
            tc.tile_pool(name="stg", bufs=2) as stgp,
            tc.tile_pool(name="persist", bufs=1) as persist,
            tc.tile_pool(name="ps", bufs=2, space="PSUM") as ps,
            tc.tile_pool(name="pstat", bufs=1, space="PSUM") as pstat,
            tc.tile_pool(name="dram", bufs=1, space="DRAM") as dram,
        ):
            ident = constp.tile([P, P], f32)
            make_identity(nc, ident[:])
            ones_f = constp.tile([P, 1], f32)
            nc.vector.memset(ones_f[:], 1.0)
            ones_b = constp.tile([P, 1], bf16)
            nc.vector.memset(ones_b[:], 1.0)

            wstk_t = constp.tile([D, 2 * 67], bf16)
            nc.sync.dma_start(
                out=wstk_t[:].rearrange("k (l n) -> k l n", n=67),
                in_=wstk_in[:, :, :].rearrange("l k n -> k l n"))

            padrow = constp.tile([1, ROW], bf16)
            nc.vector.memset(padrow[:], 0.0)
            nc.vector.memset(padrow[:, C_EL:C_EL + 1], PAD_EL)

            # idx streams resident (identical across layers)
            idx_sb = persist.tile([P, C16], i16)
            nc.sync.dma_start(out=idx_sb[:], in_=idx_in[:, :])

            er_sb = persist.tile([P, N_TILES], f32)
            nc.sync.dma_start(out=er_sb[:], in_=er1_in[:, :])

            out_sb = persist.tile([P, N_TILES * D], f32)

            tab1 = dram.tile([TAB_ROWS, ROW], bf16, name="tab1")
            tab2 = dram.tile([TAB_ROWS, ROW], bf16, name="tab2")
            tabs = [tab0_in, tab1, tab2]
            shard_buf = dram.tile([SHARD_P, ROW], bf16)
            stats_dram_in = dram.tile([D, 2], f32)
            stats_dram_out = dram.tile([D, 2], f32)
            bcast_dram = dram.tile([3, D], f32)

            red_ctr = 0
            mul_ctr = 0
            for l in range(N_LAYERS):
                tab = tabs[l]
                win = [tab[w * WIN_ROWS:(w + 1) * WIN_ROWS, :] for w in range(N_WIN)]
                stat_s = pstat.tile([D, 1], f32, tag="stat_s")
                stat_q = pstat.tile([D, 1], f32, tag="stat_q")

                for ci, ch in enumerate(chunks):
                    nt = len(ch)
                    nWs = [int(W_tw[ch, w].sum()) for w in range(N_WIN)]
                    act_w = [w for w in range(N_WIN) if nWs[w] > 0]
                    nW_tot = sum(nWs)
                    ow = {}
                    o = 0
                    for w in act_w:
                        ow[w] = o
                        o += nWs[w]

                    # per window: gather -> prelu -> exp -> mult -> reduces
                    acc = accp.tile([P, nt * N_WIN * 65], f32, tag="acc")
                    for w in act_w:
                        n_idx, off16 = meta_by_cw[(ci, w)]
                        gt = gbuf.tile([P, nWs[w] * ROW], bf16, tag="g")
                        nc.gpsimd.dma_gather(
                            out_ap=gt[:].rearrange("p (c r) -> p c r", r=ROW),
                            in_ap=win[w][:, :] if l == 0 else win[w],
                            idxs_ap=idx_sb[:, off16: off16 + n_idx // 16],
                            num_idxs=n_idx, num_idxs_reg=n_idx,
                            elem_size=ROW, single_packet=False,
                        )
                        g3 = gt[:].rearrange("p (c r) -> p c r", r=ROW)

                        # e = lrelu(el + er) per tile
                        e_w = small.tile([P, nWs[w]], bf16, tag="e")
                        o = 0
                        for t in ch:
                            Wt = int(W_tw[t, w])
                            if Wt == 0:
                                continue
                            el_v = g3[:, o:o + Wt, C_EL:C_EL + 1].rearrange(
                                "p w o -> p (w o)")
                            dst_sl = e_w[:, o:o + Wt]
                            if SIM_SAFE:
                                nc.scalar.activation(
                                    out=dst_sl, in_=el_v,
                                    func=mybir.ActivationFunctionType.Identity,
                                    bias=er_sb[:, t:t + 1], scale=1.0)
                                e_s = small.tile([P, Wt], bf16, tag="es")
                                nc.vector.tensor_scalar(
                                    out=e_s[:], in0=dst_sl, scalar1=NEG_SLOPE,
                                    scalar2=None, op0=mybir.AluOpType.mult)
                                nc.vector.tensor_tensor(
                                    out=dst_sl, in0=dst_sl, in1=e_s[:],
                                    op=mybir.AluOpType.max)
                            else:
                                nc.scalar.activation(
                                    out=dst_sl, in_=el_v,
                                    func=mybir.ActivationFunctionType.Prelu,
                                    bias=er_sb[:, t:t + 1], scale=1.0,
                                    alpha=NEG_SLOPE)
                            o += Wt

                        # exp materialized broadcast over 65 cols (ACT) so the
                        # DVE mult sees all-packed bf16 aps -> 2x mode
                        exm = xmp.tile([P, nWs[w] * 65], bf16, tag="xm")
                        exm3 = exm[:].rearrange("p (c r) -> p c r", r=65)
                        nc.scalar.activation(
                            out=exm3,
                            in_=e_w[:].unsqueeze(2).to_broadcast([P, nWs[w], 65]),
                            func=mybir.ActivationFunctionType.Exp)

                        wf = wfp.tile([P, nWs[w] * 65], bf16, tag="wf")
                        wf3 = wf[:].rearrange("p (c r) -> p c r", r=65)
                        mul_ctr += 1
                        meng = nc.gpsimd if (MUL_POOL_MOD and
                                             mul_ctr % MUL_POOL_MOD == 0) \
                            else nc.vector
                        meng.tensor_tensor(
                            out=wf3, in0=g3[:, :, 0:65], in1=exm3,
                            op=mybir.AluOpType.mult)
                        o = 0
                        for ti, t in enumerate(ch):
                            Wt = int(W_tw[t, w])
                            if Wt == 0:
                                continue
                            red_ctr += 1
                            nc.vector.tensor_reduce(
                                out=acc[:, (ti * N_WIN + w) * 65:
                                        (ti * N_WIN + w + 1) * 65],
                                in_=wf3[:, o:o + Wt, :].rearrange("p w d -> p d w"),
                                axis=mybir.AxisListType.X, op=mybir.AluOpType.add)
                            o += Wt

                    # combine windows per tile -> comb [P, nt*65]
                    comb = small.tile([P, nt * 65], f32, tag="cb")
                    for ti, t in enumerate(ch):
                        aws = [w for w in range(N_WIN) if W_tw[t, w] > 0]
                        base = ti * N_WIN
                        dst = comb[:, ti * 65:(ti + 1) * 65]
                        if len(aws) == 1:
                            nc.vector.tensor_copy(
                                out=dst, in_=acc[:, (base + aws[0]) * 65:
                                                 (base + aws[0] + 1) * 65])
                        elif len(aws) == N_WIN:
                            nc.vector.tensor_reduce(
                                out=dst,
                                in_=acc[:, base * 65:(base + N_WIN) * 65]
                                .rearrange("p (w d) -> p d w", d=65),
                                axis=mybir.AxisListType.X,
                                op=mybir.AluOpType.add)
                        else:
                            nc.vector.tensor_copy(
                                out=dst, in_=acc[:, (base + aws[0]) * 65:
                                                 (base + aws[0] + 1) * 65])
                            for w in aws[1:]:
                                nc.vector.tensor_tensor(
                                    out=dst, in0=dst,
                                    in1=acc[:, (base + w) * 65:
                                            (base + w + 1) * 65],
                                    op=mybir.AluOpType.add)

                    # divide by denominator (col 64) -> out_sb
                    rinv = small.tile([P, nt], f32, tag="ri")
                    nc.vector.reciprocal(
                        out=rinv[:],
                        in_=comb[:].rearrange("p (t d) -> p d t", d=65)[:, C_ONE, :])
                    sq = small.tile([P, nt * D], bf16, tag="sq")
                    for ti, t in enumerate(ch):
                        nc.scalar.activation(
                            out=out_sb[:, t * D:(t + 1) * D],
                            in_=comb[:, ti * 65: ti * 65 + D],
                            func=mybir.ActivationFunctionType.Copy,
                            scale=rinv[:, ti:ti + 1])
                    nc.scalar.activation(
                        out=sq[:], in_=out_sb[:, ch[0] * D:(ch[0] + nt) * D],
                        func=mybir.ActivationFunctionType.Square)
                    for ti, t in enumerate(ch):
                        m = P if t < N_TILES - 1 else LAST_TILE_N
                        first = (t == 0)
                        last = (t == N_TILES - 1)
                        nc.tensor.matmul(out=stat_s[:, :],
                                         lhsT=out_sb[:m, t * D:(t + 1) * D],
                                         rhs=ones_f[:m, :],
                                         start=first, stop=last)
                        nc.tensor.matmul(out=stat_q[:, :],
                                         lhsT=sq[:m, ti * D:(ti + 1) * D],
                                         rhs=ones_b[:m, :],
                                         start=first, stop=last)

                # ---- BN stats all-reduce + coefficients ----
                stat_sb = small.tile([D, 2], f32, tag="stc")
                nc.vector.tensor_copy(out=stat_sb[:, 0:1], in_=stat_s[:])
                nc.vector.tensor_copy(out=stat_sb[:, 1:2], in_=stat_q[:])
                nc.sync.dma_start(out=stats_dram_in[:], in_=stat_sb[:])
                if not NO_COLL:
                    nc.gpsimd.collective_compute(
                        "AllReduce", mybir.AluOpType.add, replica_groups=rg,
                        ins=[stats_dram_in.opt()], outs=[stats_dram_out.opt()])
                stat_g = small.tile([D, 2], f32, tag="stg")
                nc.sync.dma_start(
                    out=stat_g[:],
                    in_=(stats_dram_in if NO_COLL else stats_dram_out)[:])
                mu = small.tile([D, 1], f32, tag="mu")
                nc.vector.tensor_scalar(out=mu[:], in0=stat_g[:, 0:1],
                                        scalar1=1.0 / N_NODES, scalar2=None,
                                        op0=mybir.AluOpType.mult)
                musq = small.tile([D, 1], f32, tag="musq")
                nc.scalar.activation(out=musq[:], in_=mu[:],
                                     func=mybir.ActivationFunctionType.Square)
                var = small.tile([D, 1], f32, tag="var")
                nc.vector.tensor_scalar(out=var[:], in0=stat_g[:, 1:2],
                                        scalar1=1.0 / N_NODES, scalar2=BN_EPS,
                                        op0=mybir.AluOpType.mult,
                                        op1=mybir.AluOpType.add)
                nc.vector.tensor_tensor(out=var[:], in0=var[:], in1=musq[:],
                                        op=mybir.AluOpType.subtract)
                sd = small.tile([D, 1], f32, tag="sd")
                nc.scalar.activation(out=sd[:], in_=var[:],
                                     func=mybir.ActivationFunctionType.Sqrt)
                rstd = small.tile([D, 1], f32, tag="rstd")
                nc.vector.reciprocal(out=rstd[:], in_=sd[:])
                gcol = small.tile([D, 1], f32, tag="gc")
                nc.sync.dma_start(out=gcol[:], in_=bnp_in[l, 0, :][:, None])
                bcol = small.tile([D, 1], f32, tag="bc")
                nc.sync.dma_start(out=bcol[:], in_=bnp_in[l, 1, :][:, None])
                grs = small.tile([D, 1], f32, tag="grs")
                nc.vector.tensor_tensor(out=grs[:], in0=gcol[:], in1=rstd[:],
                                        op=mybir.AluOpType.mult)
                negmu = small.tile([D, 1], f32, tag="nmu")
                nc.vector.tensor_scalar(out=negmu[:], in0=mu[:], scalar1=-1.0,
                                        scalar2=None, op0=mybir.AluOpType.mult)
                nc.sync.dma_start(out=bcast_dram[0, :], in_=negmu[:, 0])
                nc.sync.dma_start(out=bcast_dram[1, :], in_=grs[:, 0])
                nc.sync.dma_start(out=bcast_dram[2, :], in_=bcol[:, 0])
                brow = small.tile([P, 3 * D], f32, tag="brow")
                nc.sync.dma_start(
                    out=brow[:],
                    in_=bcast_dram[:, :].rearrange("a b -> (a b)")[None, :]
                        .to_broadcast([P, 3 * D]))

                # ---- pass 2, per chunk: BN (+ELU, project, stage) ----
                def brow_b(k, nt):
                    return (brow[:, k * D:(k + 1) * D].unsqueeze(1)
                            .to_broadcast([P, nt, D]))

                for ci, ch in enumerate(chunks):
                    nt = len(ch)
                    t0 = ch[0]
                    osb3 = out_sb[:, t0 * D:(t0 + nt) * D].rearrange(
                        "p (t d) -> p t d", d=D)
                    beng = nc.gpsimd if (l == N_LAYERS - 1 and ci % 3 == 2) \
                        else nc.vector
                    beng.tensor_tensor(out=osb3, in0=osb3,
                                       in1=brow_b(0, nt),
                                       op=mybir.AluOpType.add)
                    beng.tensor_tensor(out=osb3, in0=osb3,
                                       in1=brow_b(1, nt),
                                       op=mybir.AluOpType.mult)
                    beng.tensor_tensor(out=osb3, in0=osb3,
                                       in1=brow_b(2, nt),
                                       op=mybir.AluOpType.add)
                    if l < N_LAYERS - 1:
                        osb2 = out_sb[:, t0 * D:(t0 + nt) * D]
                        ez = small.tile([P, nt * D], bf16, tag="ez")
                        nc.scalar.activation(
                            out=ez[:], in_=osb2,
                            func=mybir.ActivationFunctionType.Exp)
                        nc.vector.tensor_scalar(
                            out=ez[:], in0=ez[:], scalar1=-1.0, scalar2=0.0,
                            op0=mybir.AluOpType.add, op1=mybir.AluOpType.min)
                        nc.vector.tensor_scalar(
                            out=osb2, in0=osb2, scalar1=0.0, scalar2=None,
                            op0=mybir.AluOpType.max)
                        nc.vector.tensor_tensor(out=osb2, in0=osb2,
                                                in1=ez[:],
                                                op=mybir.AluOpType.add)
                        stage = stgp.tile([P, nt * 66], bf16, tag="st")
                        for ti, t in enumerate(ch):
                            m = P if t < N_TILES - 1 else LAST_TILE_N
                            pT = ps.tile([D, P], f32, tag="pT")
                            nc.tensor.transpose(
                                out=pT[:, :m], in_=out_sb[:m, t * D:(t + 1) * D],
                                identity=ident[:m, :m])
                            h2 = small.tile([D, P], bf16, tag="h2")
                            nc.vector.tensor_copy(out=h2[:, :m], in_=pT[:, :m])
                            pj = ps.tile([P, 67], f32, tag="pj")
                            nc.tensor.matmul(
                                out=pj[:m, :], lhsT=h2[:, :m],
                                rhs=wstk_t[:, l * 67:(l + 1) * 67],
                                start=True, stop=True)
                            nc.scalar.copy(out=stage[:m, ti * 66:ti * 66 + 66],
                                           in_=pj[:m, 0:66])
                            nc.vector.memset(
                                stage[:, ti * 66 + C_ONE: ti * 66 + C_ONE + 1],
                                1.0)
                            nc.vector.tensor_copy(out=er_sb[:m, t:t + 1],
                                                  in_=pj[:m, 66:67])
                        if ch[-1] < N_TILES - 1:
                            nc.sync.dma_start(
                                out=shard_buf[t0 * P: t0 * P + nt * P, 0:66]
                                    .rearrange("(j p) n -> p j n", p=P),
                                in_=stage[:].rearrange("p (j n) -> p j n", n=66))
                        else:
                            if nt > 1:
                                nc.sync.dma_start(
                                    out=shard_buf[t0 * P: t0 * P + (nt - 1) * P, 0:66]
                                        .rearrange("(j p) n -> p j n", p=P),
                                    in_=stage[:, :(nt - 1) * 66]
                                        .rearrange("p (j n) -> p j n", n=66))
                            tl = ch[-1]
                            nc.sync.dma_start(
                                out=shard_buf[tl * P: tl * P + LAST_TILE_N, 0:66],
                                in_=stage[:LAST_TILE_N, (nt - 1) * 66: nt * 66])
                    else:
                        nc.sync.dma_start(
                            out=out_t[:, t0 * D:(t0 + nt) * D],
                            in_=out_sb[:, t0 * D:(t0 + nt) * D])
                if l < N_LAYERS - 1:
                    nc.sync.dma_start(out=shard_buf[SHARD:SHARD + 1, :],
                                      in_=padrow[:, :])
                    if not NO_COLL:
                        nc.gpsimd.collective_compute(
                            "AllGather", mybir.AluOpType.bypass,
                            replica_groups=rg,
                            ins=[shard_buf.opt()], outs=[tabs[l + 1].opt()])

    nc.compile()
    return nc


_CACHE = {}


def kernel(node_weight, edge_weight, src, dst,
           W1, al1, ar1, b1, g1, beta1,
           W2, al2, ar2, b2, g2, beta2,
           W3, al3, ar3, b3, g3, beta3):
    Ws = [np.asarray(W1, np.float32), np.asarray(W2, np.float32), np.asarray(W3, np.float32)]
    als = [np.asarray(al1, np.float32), np.asarray(al2, np.float32), np.asarray(al3, np.float32)]
    ars = [np.asarray(ar1, np.float32), np.asarray(ar2, np.float32), np.asarray(ar3, np.float32)]
    pre = _preprocess(node_weight, src, dst, Ws, als, ars)

    C16 = pre["idx_streams"][0].shape[1]
    key = ("nc", C16, N_LAYERS, NO_COLL, tuple(pre["W_tw"].reshape(-1).tolist()))
    if key not in _CACHE:
        _CACHE[key] = _build_nc(pre["W_tw"], pre["chunks"], pre["call_meta"], C16)
    nc = _CACHE[key]

    bnp = np.stack([
        np.stack([np.asarray(g, np.float32), np.asarray(be, np.float32)])
        for g, be in ((g1, beta1), (g2, beta2), (g3, beta3))
    ])  # [3, 2, 64]

    in_maps = []
    for c in range(N_CORES):
        in_maps.append({
            "tab0": pre["tab0"],
            "idx": pre["idx_streams"][c],
            "er1": pre["er1_sb"][c],
            "wstk": pre["wstk"],
            "bnp": bnp,
        })
    res = bass_utils.run_bass_kernel_spmd(nc, in_maps, core_ids=list(range(N_CORES)))

    out = np.empty((N_NODES, D), np.float32)
    for c in range(N_CORES):
        rows = pre["orig_of"][c * SHARD: (c + 1) * SHARD]
        # out_t [128, 98*64]: node (t*128+p) at [p, t*64:(t+1)*64]
        oc = np.asarray(res.results[c]["out"], np.float32)
        oc = oc.reshape(P, N_TILES, D).transpose(1, 0, 2).reshape(-1, D)
        out[rows] = oc[:SHARD]
    return out
